# revision 34
# baseline (speedup 1.0000x reference)
"""Talking-heads attention on 8 Trainium2 NeuronCores.

Sharding: data-parallel over (batch b in 0..3) x (query half in 0..1) -> 8 cores.
Each core computes K/V for its full batch sequence (1024) and attention for its
512 query rows. No collectives.

Math notes (per core, all layouts transposed so contractions sit on partitions):
  - mix_pre (and SCALE) folded into Q per output-head g:
    qs_g[hd, i] = qT[hd, i] * scaleT[hd, g], scaleT = SCALE*mix_pre[h(hd), g]
    (host-precomputed), so dotsT_g[j, i] = sum_hd kT[hd, j] * qs_g[hd, i].
  - softmax over j (partitions) without max-subtraction (|dots| <~ 6, safe).
    The denominator never touches the PE: DVE tree-sums the 8 key chunks,
    GPSIMD partition_all_reduce sums across partitions (result broadcast to
    all partitions), DVE reciprocal + in-place scale of attnT.
  - mix_post folded into V: Vt_g[j, (g',d)] = mix_post[g, g'] * v[j, (g',d)];
    out2T[(g'd), i] += sum_j Vt_g[j, gd] * attnT_g[j, i] accumulated in PSUM
    over g (6 banks) while dots for g+2 stream (lag-2 software pipeline).
  - out = out2T.T @ Wout + bout (bf16), bias folded in as a K=1 accumulating
    matmul.
Scheduling notes: PE runs one long matmul stream (projections, 12x dots,
12x AV, out-proj); qs/Vt for the first heads and all PSUM->SBUF staging are
produced in phase 1 where ACT/DVE are idle, so the fill iterations of the
g-loop never stall on the scalar engines.
"""

import numpy as np

import concourse.bass_isa as bass_isa
import concourse.mybir as mybir
import concourse.tile as tile
from concourse import bacc
from concourse.bass_utils import run_bass_kernel_spmd

P = 128
DIM = 768
SEQ = 1024
IQ = 512            # query rows per core
H = 12
DH = 64
NC6 = DIM // P      # 6 chunks of the 768 dim
JC8 = SEQ // P      # 8 chunks of the key dim
SCALE = DH ** -0.5
F32 = mybir.dt.float32
BF16 = mybir.dt.bfloat16
EXP = mybir.ActivationFunctionType.Exp
ADD = mybir.AluOpType.add
MULT = mybir.AluOpType.mult

_CACHE = {}


def _build_nc():
    nc = bacc.Bacc("TRN2", target_bir_lowering=False, debug=False)

    xqT = nc.dram_tensor("xqT", [DIM, IQ], BF16, kind="ExternalInput")
    xkvT = nc.dram_tensor("xkvT", [DIM, SEQ], BF16, kind="ExternalInput")
    Wq = nc.dram_tensor("Wq", [DIM, DIM], BF16, kind="ExternalInput")
    Wk = nc.dram_tensor("Wk", [DIM, DIM], BF16, kind="ExternalInput")
    Wv = nc.dram_tensor("Wv", [DIM, DIM], BF16, kind="ExternalInput")
    Wout = nc.dram_tensor("Wout", [DIM, DIM], BF16, kind="ExternalInput")
    bout = nc.dram_tensor("bout", [1, DIM], BF16, kind="ExternalInput")
    scaleT_d = nc.dram_tensor("scaleT", [P, NC6 * H], F32, kind="ExternalInput")
    m2_d = nc.dram_tensor("m2", [1, H * H], F32, kind="ExternalInput")
    out = nc.dram_tensor("out", [IQ, DIM], F32, kind="ExternalOutput")

    r3 = lambda t: t.rearrange("(c p) e -> p c e", p=P)

    with tile.TileContext(nc) as tc:
        with (
            tc.tile_pool(name="persist", bufs=1) as pp,
            tc.tile_pool(name="attnp", bufs=3) as attnp,
            tc.tile_pool(name="qsp", bufs=4) as qsp,
            tc.tile_pool(name="vtp", bufs=3) as vtp,
            tc.tile_pool(name="sm1", bufs=1) as sm1,
            tc.tile_pool(name="sm2", bufs=2) as sm2,
        ):
            # ---- persistent tiles ----
            qT = pp.tile([P, NC6, IQ], BF16)
            kT = pp.tile([P, NC6, SEQ], BF16)
            V = pp.tile([P, JC8, DIM], BF16)     # [j-part, jc, (g,d)]
            Wout_sb = pp.tile([P, NC6, DIM], BF16)
            o2_sb = pp.tile([P, NC6, IQ], BF16)  # out2T staged for out-proj
            scaleT = pp.tile([P, NC6 * H], F32)  # SCALE*mix_pre expanded
            m2_sb = pp.tile([1, H * H], F32)
            m2bc = pp.tile([P, H * H], F32)      # mix_post bcast to all parts
            bout_sb = pp.tile([1, DIM], BF16)
            ones_row = pp.tile([1, P], BF16)
            nc.gpsimd.memset(ones_row[:], 1.0)

            s1 = sm1.tile([P, 4, IQ], BF16)
            s2 = sm1.tile([P, 2, IQ], BF16)

            qss, attnTs, vts = {}, {}, {}

            def make_qs(g):
                # qs(g) on ACT: per-partition scale fold
                qs = qsp.tile([P, NC6, IQ], BF16, tag="qs", name=f"qs{g}")
                for c in range(NC6):
                    nc.scalar.mul(
                        qs[:, c, :], qT[:, c, :],
                        scaleT[:, c * H + g : c * H + g + 1],
                    )
                qss[g] = qs

            def make_vt(g):
                # Vt(g) on DVE: mix_post column fold per 64-col group
                vt = vtp.tile([P, JC8, DIM], BF16, tag="vt", name=f"vt{g}")
                for gp in range(H):
                    nc.vector.tensor_scalar_mul(
                        vt[:, :, gp * DH : (gp + 1) * DH],
                        V[:, :, gp * DH : (gp + 1) * DH],
                        m2bc[:, g * H + gp : g * H + gp + 1],
                    )
                vts[g] = vt

            # ---- phase 1: input DMA + projections; also pre-produce the
            # qs/Vt operands the g-loop fill needs, while ACT/DVE are idle.
            with (
                tc.tile_pool(name="pin", bufs=1) as pin,
                tc.tile_pool(name="pj", bufs=3, space="PSUM") as pj,
                tc.tile_pool(name="pjv", bufs=2, space="PSUM") as pjv,
            ):
                xqT_sb = pin.tile([P, NC6, IQ], BF16)
                Wq_sb = pin.tile([P, NC6, DIM], BF16)
                xkvT_sb = pin.tile([P, NC6, SEQ], BF16)
                Wk_sb = pin.tile([P, NC6, DIM], BF16)
                Wv_sb = pin.tile([P, NC6, DIM], BF16)
                # input DMA: one queue, in consumption order; Wq/xqT land as
                # halves so the q projection can start on partial data.
                nc.sync.dma_start(Wq_sb[:, 0:3, :], r3(Wq)[:, 0:3, :])
                nc.sync.dma_start(xqT_sb[:, 0:3, :], r3(xqT)[:, 0:3, :])
                nc.sync.dma_start(Wq_sb[:, 3:6, :], r3(Wq)[:, 3:6, :])
                nc.sync.dma_start(xqT_sb[:, 3:6, :], r3(xqT)[:, 3:6, :])
                nc.sync.dma_start(xkvT_sb[:], r3(xkvT))
                nc.sync.dma_start(Wk_sb[:], r3(Wk))
                nc.sync.dma_start(Wv_sb[:], r3(Wv))
                nc.gpsimd.dma_start(scaleT[:], scaleT_d[:])
                nc.gpsimd.dma_start(m2_sb[:], m2_d[:])
                nc.gpsimd.dma_start(bout_sb[:], bout[:])
                nc.gpsimd.partition_broadcast(m2bc[:], m2_sb[:])

                # qT[e,i] = sum_f Wq[f,e] xqT[f,i]
                for ec in range(NC6):
                    ps = pj.tile([P, IQ], F32, tag="pjq")
                    for fc in range(NC6):
                        nc.tensor.matmul(
                            ps[:], Wq_sb[:, fc, ec * P : (ec + 1) * P],
                            xqT_sb[:, fc, :], start=(fc == 0), stop=(fc == NC6 - 1),
                        )
                    nc.scalar.copy(qT[:, ec, :], ps[:])

                # first two heads' qs while ACT is otherwise idle
                make_qs(0)
                make_qs(1)

                # kT[e,j]
                for ec in range(NC6):
                    for jh in range(2):
                        ps = pj.tile([P, IQ], F32, tag="pjq")
                        for fc in range(NC6):
                            nc.tensor.matmul(
                                ps[:], Wk_sb[:, fc, ec * P : (ec + 1) * P],
                                xkvT_sb[:, fc, jh * IQ : (jh + 1) * IQ],
                                start=(fc == 0), stop=(fc == NC6 - 1),
                            )
                        nc.scalar.copy(kT[:, ec, jh * IQ : (jh + 1) * IQ], ps[:])

                make_qs(2)

                # V[j, gd] = sum_f xkvT[f, j] Wv[f, gd]
                for jc in range(JC8):
                    psv = pjv.tile([P, DIM], F32, tag="pjv")
                    for ns, ne in ((0, 512), (512, DIM)):
                        for fc in range(NC6):
                            nc.tensor.matmul(
                                psv[:, ns:ne],
                                xkvT_sb[:, fc, jc * P : (jc + 1) * P],
                                Wv_sb[:, fc, ns:ne],
                                start=(fc == 0), stop=(fc == NC6 - 1),
                            )
                    nc.scalar.copy(V[:, jc, :], psv[:])

                # first two heads' Vt while DVE is otherwise idle
                make_vt(0)
                make_vt(1)

            # ---- phase 2: attention, lag-2 pipelined; softmax denom off-PE
            def produce(g, dspool):
                qs = qss.pop(g)
                if g >= 2:
                    make_vt(g)
                # dots(g) on PE + exp on ACT
                attnT = attnp.tile([P, JC8, IQ], BF16, tag="attnT")
                attnTs[g] = attnT
                for jc in range(JC8):
                    ds = dspool.tile([P, IQ], F32, tag="ds")
                    for c in range(NC6):
                        nc.tensor.matmul(
                            ds[:], kT[:, c, jc * P : (jc + 1) * P],
                            qs[:, c, :],
                            start=(c == 0), stop=(c == NC6 - 1),
                        )
                    nc.scalar.activation(attnT[:, jc, :], ds[:], EXP)
                # softmax denominator: DVE jc-tree + GPSIMD allreduce
                Ssum = sm2.tile([P, IQ], F32, tag="Ssum")
                rR = sm2.tile([P, IQ], F32, tag="rR")
                rRb = sm2.tile([P, IQ], BF16, tag="rRb")
                nc.vector.tensor_tensor(
                    s1[:], attnT[:, 0:4, :], attnT[:, 4:8, :], ADD
                )
                nc.vector.tensor_tensor(s2[:], s1[:, 0:2, :], s1[:, 2:4, :], ADD)
                nc.vector.tensor_tensor(Ssum[:], s2[:, 0, :], s2[:, 1, :], ADD)
                nc.gpsimd.partition_all_reduce(
                    Ssum[:], Ssum[:], channels=P, reduce_op=bass_isa.ReduceOp.add
                )
                nc.vector.reciprocal_approx_fast(rR[:], Ssum[:])
                nc.vector.tensor_copy(rRb[:], rR[:])
                nc.vector.tensor_tensor(
                    attnT[:], attnT[:],
                    rRb[:, None, :].to_broadcast((P, JC8, IQ)), MULT,
                )
                if g + 3 < H:
                    make_qs(g + 3)

            # fill iterations run before the o2 accumulator pool exists,
            # so the dots scratch gets 4 PSUM banks (no exp-lag stalls)
            with tc.tile_pool(name="pds0", bufs=4, space="PSUM") as pds0:
                produce(0, pds0)
                nc.gpsimd.dma_start(Wout_sb[:], r3(Wout))
                produce(1, pds0)

            with (
                tc.tile_pool(name="acc", bufs=1, space="PSUM") as acc,
                tc.tile_pool(name="pds", bufs=2, space="PSUM") as pds,
            ):
                o2ps = [
                    acc.tile([P, IQ], F32, tag=f"o2_{s}", name=f"o2_{s}")
                    for s in range(NC6)
                ]
                for it in range(2, H + 2):
                    if it < H:
                        produce(it, pds)
                    g2 = it - 2
                    last = g2 == H - 1
                    for s in range(NC6):
                        for jc in range(JC8):
                            nc.tensor.matmul(
                                o2ps[s][:],
                                vts[g2][:, jc, s * P : (s + 1) * P],
                                attnTs[g2][:, jc, :],
                                start=(g2 == 0 and jc == 0),
                                stop=(last and jc == JC8 - 1),
                            )
                        if last:
                            # drain each o2 bank as its final matmul lands
                            if s % 2 == 0:
                                nc.vector.tensor_copy(o2_sb[:, s, :], o2ps[s][:])
                            else:
                                nc.scalar.copy(o2_sb[:, s, :], o2ps[s][:])
                    del attnTs[g2], vts[g2]

            # ---- phase 3: output projection + bias via K=1 matmul ----
            with (
                tc.tile_pool(name="pj3", bufs=2, space="PSUM") as pj3,
                tc.tile_pool(name="ob", bufs=2) as ob,
            ):
                for isl in range(IQ // P):
                    fp = pj3.tile([P, DIM], F32, tag="fin")
                    for ns, ne in ((0, 512), (512, DIM)):
                        for c in range(NC6):
                            nc.tensor.matmul(
                                fp[:, ns:ne],
                                o2_sb[:, c, isl * P : (isl + 1) * P],
                                Wout_sb[:, c, ns:ne],
                                start=(c == 0), stop=False,
                            )
                        nc.tensor.matmul(
                            fp[:, ns:ne],
                            ones_row[:],
                            bout_sb[:, ns:ne],
                            start=False, stop=True,
                        )
                    osb = ob.tile([P, DIM], F32, tag="osb")
                    nc.scalar.copy(osb[:], fp[:])
                    eng = nc.gpsimd if isl % 2 == 0 else nc.sync
                    eng.dma_start(out[isl * P : (isl + 1) * P, :], osb[:])

    nc.compile()
    return nc


def kernel(x, Wq, Wkv, mix_pre, mix_post, Wout, bout):
    x = np.asarray(x, dtype=np.float32)
    Wq = np.asarray(Wq, dtype=np.float32)
    Wkv = np.asarray(Wkv, dtype=np.float32)
    mix_pre = np.asarray(mix_pre, dtype=np.float32)
    mix_post = np.asarray(mix_post, dtype=np.float32)
    Wout = np.asarray(Wout, dtype=np.float32)
    bout = np.asarray(bout, dtype=np.float32)

    if "nc" not in _CACHE:
        _CACHE["nc"] = _build_nc()
    nc = _CACHE["nc"]

    import ml_dtypes
    bf = ml_dtypes.bfloat16
    Wk = np.ascontiguousarray(Wkv[:, :DIM]).astype(bf)
    Wv = np.ascontiguousarray(Wkv[:, DIM:]).astype(bf)
    # scaleT[p, c*H+g] = SCALE * mix_pre[(c*128+p)//64, g]
    SM = SCALE * mix_pre[np.arange(DIM) // DH, :]          # [768, 12]
    scaleT = np.ascontiguousarray(
        SM.reshape(NC6, P, H).transpose(1, 0, 2).reshape(P, NC6 * H)
    ).astype(np.float32)
    shared = {
        "Wq": Wq.astype(bf), "Wk": Wk, "Wv": Wv, "Wout": Wout.astype(bf),
        "bout": np.ascontiguousarray(bout.reshape(1, DIM)).astype(bf),
        "scaleT": scaleT,
        "m2": np.ascontiguousarray(mix_post.reshape(1, H * H)),
    }
    b_, n_, d_ = x.shape
    in_maps = []
    for c in range(8):
        b, half = c // 2, c % 2
        m = dict(shared)
        m["xqT"] = np.ascontiguousarray(x[b, half * IQ : (half + 1) * IQ, :].T).astype(bf)
        m["xkvT"] = np.ascontiguousarray(x[b].T).astype(bf)
        in_maps.append(m)

    res = run_bass_kernel_spmd(nc, in_maps, core_ids=list(range(8)))
    _CACHE["last_results"] = res
    _CACHE["last_in_maps"] = in_maps

    full = np.empty((b_, n_, d_), dtype=np.float32)
    for c in range(8):
        b, half = c // 2, c % 2
        full[b, half * IQ : (half + 1) * IQ, :] = res.results[c]["out"]
    return full


# revision 35
# speedup vs baseline: 1.0087x; 1.0087x over previous
"""Talking-heads attention on 8 Trainium2 NeuronCores.

Sharding: data-parallel over (batch b in 0..3) x (query half in 0..1) -> 8 cores.
Each core computes K/V for its full batch sequence (1024) and attention for its
512 query rows. No collectives.

Math notes (per core, all layouts transposed so contractions sit on partitions):
  - mix_pre (and SCALE) folded into Q per output-head g:
    qs_g[hd, i] = qT[hd, i] * scaleT[hd, g], scaleT = SCALE*mix_pre[h(hd), g]
    (host-precomputed), so dotsT_g[j, i] = sum_hd kT[hd, j] * qs_g[hd, i].
  - softmax over j (partitions) without max-subtraction (|dots| <~ 6, safe).
    The denominator never touches the PE: DVE tree-sums the 8 key chunks,
    GPSIMD partition_all_reduce sums across partitions (result broadcast to
    all partitions), DVE reciprocal + in-place scale of attnT.
  - mix_post folded into V: Vt_g[j, (g',d)] = mix_post[g, g'] * v[j, (g',d)];
    out2T[(g'd), i] += sum_j Vt_g[j, gd] * attnT_g[j, i] accumulated in PSUM
    over g (6 banks) while dots for g+2 stream (lag-2 software pipeline).
  - out = out2T.T @ Wout + bout (bf16), bias folded in as a K=1 accumulating
    matmul.
Scheduling notes: PE runs one long matmul stream (projections, 12x dots,
12x AV, out-proj); qs/Vt for the first heads and all PSUM->SBUF staging are
produced in phase 1 where ACT/DVE are idle, so the fill iterations of the
g-loop never stall on the scalar engines.
"""

import numpy as np

import concourse.bass_isa as bass_isa
import concourse.mybir as mybir
import concourse.tile as tile
from concourse import bacc
from concourse.bass_utils import run_bass_kernel_spmd

P = 128
DIM = 768
SEQ = 1024
IQ = 512            # query rows per core
H = 12
DH = 64
NC6 = DIM // P      # 6 chunks of the 768 dim
JC8 = SEQ // P      # 8 chunks of the key dim
SCALE = DH ** -0.5
F32 = mybir.dt.float32
BF16 = mybir.dt.bfloat16
EXP = mybir.ActivationFunctionType.Exp
ADD = mybir.AluOpType.add
MULT = mybir.AluOpType.mult

_CACHE = {}


def _build_nc():
    nc = bacc.Bacc("TRN2", target_bir_lowering=False, debug=False)

    xqT = nc.dram_tensor("xqT", [DIM, IQ], BF16, kind="ExternalInput")
    xkvT = nc.dram_tensor("xkvT", [DIM, SEQ], BF16, kind="ExternalInput")
    Wq = nc.dram_tensor("Wq", [DIM, DIM], BF16, kind="ExternalInput")
    Wk = nc.dram_tensor("Wk", [DIM, DIM], BF16, kind="ExternalInput")
    Wv = nc.dram_tensor("Wv", [DIM, DIM], BF16, kind="ExternalInput")
    Wout = nc.dram_tensor("Wout", [DIM, DIM], BF16, kind="ExternalInput")
    bout = nc.dram_tensor("bout", [1, DIM], BF16, kind="ExternalInput")
    scaleT_d = nc.dram_tensor("scaleT", [P, NC6 * H], F32, kind="ExternalInput")
    m2_d = nc.dram_tensor("m2", [1, H * H], F32, kind="ExternalInput")
    out = nc.dram_tensor("out", [IQ, DIM], F32, kind="ExternalOutput")

    r3 = lambda t: t.rearrange("(c p) e -> p c e", p=P)

    with tile.TileContext(nc) as tc:
        with (
            tc.tile_pool(name="persist", bufs=1) as pp,
            tc.tile_pool(name="attnp", bufs=3) as attnp,
            tc.tile_pool(name="qsp", bufs=4) as qsp,
            tc.tile_pool(name="vtp", bufs=3) as vtp,
            tc.tile_pool(name="sm1", bufs=1) as sm1,
            tc.tile_pool(name="sm2", bufs=2) as sm2,
        ):
            # ---- persistent tiles ----
            qT = pp.tile([P, NC6, IQ], BF16)
            kT = pp.tile([P, NC6, SEQ], BF16)
            V = pp.tile([P, JC8, DIM], BF16)     # [j-part, jc, (g,d)]
            Wout_sb = pp.tile([P, NC6, DIM], BF16)
            o2_sb = pp.tile([P, NC6, IQ], BF16)  # out2T staged for out-proj
            scaleT = pp.tile([P, NC6 * H], F32)  # SCALE*mix_pre expanded
            m2_sb = pp.tile([1, H * H], F32)
            m2bc = pp.tile([P, H * H], F32)      # mix_post bcast to all parts
            bout_sb = pp.tile([1, DIM], BF16)
            ones_row = pp.tile([1, P], BF16)
            nc.gpsimd.memset(ones_row[:], 1.0)

            s1 = sm1.tile([P, 4, IQ], BF16)
            s2 = sm1.tile([P, 2, IQ], BF16)

            qss, attnTs, vts = {}, {}, {}

            def make_qs(g):
                # qs(g) on ACT: per-partition scale fold
                qs = qsp.tile([P, NC6, IQ], BF16, tag="qs", name=f"qs{g}")
                for c in range(NC6):
                    nc.scalar.mul(
                        qs[:, c, :], qT[:, c, :],
                        scaleT[:, c * H + g : c * H + g + 1],
                    )
                qss[g] = qs

            def make_vt(g):
                # Vt(g) on DVE: mix_post column fold per 64-col group
                vt = vtp.tile([P, JC8, DIM], BF16, tag="vt", name=f"vt{g}")
                for gp in range(H):
                    nc.vector.tensor_scalar_mul(
                        vt[:, :, gp * DH : (gp + 1) * DH],
                        V[:, :, gp * DH : (gp + 1) * DH],
                        m2bc[:, g * H + gp : g * H + gp + 1],
                    )
                vts[g] = vt

            # ---- phase 1: input DMA + projections; also pre-produce the
            # qs/Vt operands the g-loop fill needs, while ACT/DVE are idle.
            with (
                tc.tile_pool(name="pin", bufs=1) as pin,
                tc.tile_pool(name="pj", bufs=2, space="PSUM") as pj,
                tc.tile_pool(name="pjv", bufs=2, space="PSUM") as pjv,
            ):
                xqT_sb = pin.tile([P, NC6, IQ], BF16)
                Wq_sb = pin.tile([P, NC6, DIM], BF16)
                xkvT_sb = pin.tile([P, NC6, SEQ], BF16)
                Wk_sb = pin.tile([P, NC6, DIM], BF16)
                Wv_sb = pin.tile([P, NC6, DIM], BF16)
                # input DMA: one queue, in consumption order; Wq/xqT land as
                # halves so the q projection can start on partial data.
                nc.sync.dma_start(Wq_sb[:, 0:3, :], r3(Wq)[:, 0:3, :])
                nc.sync.dma_start(xqT_sb[:, 0:3, :], r3(xqT)[:, 0:3, :])
                nc.sync.dma_start(Wq_sb[:, 3:6, :], r3(Wq)[:, 3:6, :])
                nc.sync.dma_start(xqT_sb[:, 3:6, :], r3(xqT)[:, 3:6, :])
                nc.sync.dma_start(xkvT_sb[:], r3(xkvT))
                nc.sync.dma_start(Wk_sb[:], r3(Wk))
                nc.sync.dma_start(Wv_sb[:], r3(Wv))
                nc.gpsimd.dma_start(scaleT[:], scaleT_d[:])
                nc.gpsimd.dma_start(m2_sb[:], m2_d[:])
                nc.gpsimd.dma_start(bout_sb[:], bout[:])
                nc.gpsimd.partition_broadcast(m2bc[:], m2_sb[:])

                # qT[e,i] = sum_f Wq[f,e] xqT[f,i]
                for ec in range(NC6):
                    ps = pj.tile([P, IQ], F32, tag="pjq")
                    for fc in range(NC6):
                        nc.tensor.matmul(
                            ps[:], Wq_sb[:, fc, ec * P : (ec + 1) * P],
                            xqT_sb[:, fc, :], start=(fc == 0), stop=(fc == NC6 - 1),
                        )
                    nc.scalar.copy(qT[:, ec, :], ps[:])

                # first two heads' qs while ACT is otherwise idle
                make_qs(0)
                make_qs(1)

                # kT[e,j]
                for ec in range(NC6):
                    for jh in range(2):
                        ps = pj.tile([P, IQ], F32, tag="pjq")
                        for fc in range(NC6):
                            nc.tensor.matmul(
                                ps[:], Wk_sb[:, fc, ec * P : (ec + 1) * P],
                                xkvT_sb[:, fc, jh * IQ : (jh + 1) * IQ],
                                start=(fc == 0), stop=(fc == NC6 - 1),
                            )
                        nc.scalar.copy(kT[:, ec, jh * IQ : (jh + 1) * IQ], ps[:])

                make_qs(2)

                # V[j, gd] = sum_f xkvT[f, j] Wv[f, gd]
                for jc in range(JC8):
                    psv = pjv.tile([P, DIM], F32, tag="pjv")
                    for ns, ne in ((0, 512), (512, DIM)):
                        for fc in range(NC6):
                            nc.tensor.matmul(
                                psv[:, ns:ne],
                                xkvT_sb[:, fc, jc * P : (jc + 1) * P],
                                Wv_sb[:, fc, ns:ne],
                                start=(fc == 0), stop=(fc == NC6 - 1),
                            )
                    nc.scalar.copy(V[:, jc, :], psv[:])

                # first two heads' Vt while DVE is otherwise idle
                make_vt(0)
                make_vt(1)

            # ---- phase 2: attention, lag-2 pipelined; softmax denom off-PE
            def produce(g, dspool):
                qs = qss.pop(g)
                if g >= 2:
                    make_vt(g)
                # dots(g) on PE + exp on ACT
                attnT = attnp.tile([P, JC8, IQ], BF16, tag="attnT")
                attnTs[g] = attnT
                for jc in range(JC8):
                    ds = dspool.tile([P, IQ], F32, tag="ds")
                    for c in range(NC6):
                        nc.tensor.matmul(
                            ds[:], kT[:, c, jc * P : (jc + 1) * P],
                            qs[:, c, :],
                            start=(c == 0), stop=(c == NC6 - 1),
                        )
                    nc.scalar.activation(attnT[:, jc, :], ds[:], EXP)
                # softmax denominator: DVE jc-tree + GPSIMD allreduce
                Ssum = sm2.tile([P, IQ], F32, tag="Ssum")
                rR = sm2.tile([P, IQ], F32, tag="rR")
                rRb = sm2.tile([P, IQ], BF16, tag="rRb")
                nc.vector.tensor_tensor(
                    s1[:], attnT[:, 0:4, :], attnT[:, 4:8, :], ADD
                )
                nc.vector.tensor_tensor(s2[:], s1[:, 0:2, :], s1[:, 2:4, :], ADD)
                nc.vector.tensor_tensor(Ssum[:], s2[:, 0, :], s2[:, 1, :], ADD)
                nc.gpsimd.partition_all_reduce(
                    Ssum[:], Ssum[:], channels=P, reduce_op=bass_isa.ReduceOp.add
                )
                nc.vector.reciprocal_approx_fast(rR[:], Ssum[:])
                nc.vector.tensor_copy(rRb[:], rR[:])
                nc.vector.tensor_tensor(
                    attnT[:], attnT[:],
                    rRb[:, None, :].to_broadcast((P, JC8, IQ)), MULT,
                )
                if g + 3 < H:
                    make_qs(g + 3)

            # fill iterations run before the o2 accumulator pool exists,
            # so the dots scratch gets 4 PSUM banks (no exp-lag stalls)
            with tc.tile_pool(name="pds0", bufs=4, space="PSUM") as pds0:
                produce(0, pds0)
                nc.gpsimd.dma_start(Wout_sb[:], r3(Wout))
                produce(1, pds0)

            with (
                tc.tile_pool(name="acc", bufs=1, space="PSUM") as acc,
                tc.tile_pool(name="pds", bufs=2, space="PSUM") as pds,
            ):
                o2ps = [
                    acc.tile([P, IQ], F32, tag=f"o2_{s}", name=f"o2_{s}")
                    for s in range(NC6)
                ]
                for it in range(2, H + 2):
                    if it < H:
                        produce(it, pds)
                    g2 = it - 2
                    last = g2 == H - 1
                    for s in range(NC6):
                        for jc in range(JC8):
                            nc.tensor.matmul(
                                o2ps[s][:],
                                vts[g2][:, jc, s * P : (s + 1) * P],
                                attnTs[g2][:, jc, :],
                                start=(g2 == 0 and jc == 0),
                                stop=(last and jc == JC8 - 1),
                            )
                        if last:
                            # drain each o2 bank as its final matmul lands
                            if s % 2 == 0:
                                nc.vector.tensor_copy(o2_sb[:, s, :], o2ps[s][:])
                            else:
                                nc.scalar.copy(o2_sb[:, s, :], o2ps[s][:])
                    del attnTs[g2], vts[g2]

            # ---- phase 3: output projection + bias via K=1 matmul ----
            with (
                tc.tile_pool(name="pj3", bufs=2, space="PSUM") as pj3,
                tc.tile_pool(name="ob", bufs=2) as ob,
            ):
                for isl in range(IQ // P):
                    fp = pj3.tile([P, DIM], F32, tag="fin")
                    for ns, ne in ((0, 512), (512, DIM)):
                        for c in range(NC6):
                            nc.tensor.matmul(
                                fp[:, ns:ne],
                                o2_sb[:, c, isl * P : (isl + 1) * P],
                                Wout_sb[:, c, ns:ne],
                                start=(c == 0), stop=False,
                            )
                        nc.tensor.matmul(
                            fp[:, ns:ne],
                            ones_row[:],
                            bout_sb[:, ns:ne],
                            start=False, stop=True,
                        )
                    osb = ob.tile([P, DIM], F32, tag="osb")
                    nc.scalar.copy(osb[:], fp[:])
                    eng = nc.gpsimd if isl % 2 == 0 else nc.sync
                    eng.dma_start(out[isl * P : (isl + 1) * P, :], osb[:])

    nc.compile()
    return nc


def kernel(x, Wq, Wkv, mix_pre, mix_post, Wout, bout):
    x = np.asarray(x, dtype=np.float32)
    Wq = np.asarray(Wq, dtype=np.float32)
    Wkv = np.asarray(Wkv, dtype=np.float32)
    mix_pre = np.asarray(mix_pre, dtype=np.float32)
    mix_post = np.asarray(mix_post, dtype=np.float32)
    Wout = np.asarray(Wout, dtype=np.float32)
    bout = np.asarray(bout, dtype=np.float32)

    if "nc" not in _CACHE:
        _CACHE["nc"] = _build_nc()
    nc = _CACHE["nc"]

    import ml_dtypes
    bf = ml_dtypes.bfloat16
    Wk = np.ascontiguousarray(Wkv[:, :DIM]).astype(bf)
    Wv = np.ascontiguousarray(Wkv[:, DIM:]).astype(bf)
    # scaleT[p, c*H+g] = SCALE * mix_pre[(c*128+p)//64, g]
    SM = SCALE * mix_pre[np.arange(DIM) // DH, :]          # [768, 12]
    scaleT = np.ascontiguousarray(
        SM.reshape(NC6, P, H).transpose(1, 0, 2).reshape(P, NC6 * H)
    ).astype(np.float32)
    shared = {
        "Wq": Wq.astype(bf), "Wk": Wk, "Wv": Wv, "Wout": Wout.astype(bf),
        "bout": np.ascontiguousarray(bout.reshape(1, DIM)).astype(bf),
        "scaleT": scaleT,
        "m2": np.ascontiguousarray(mix_post.reshape(1, H * H)),
    }
    b_, n_, d_ = x.shape
    in_maps = []
    for c in range(8):
        b, half = c // 2, c % 2
        m = dict(shared)
        m["xqT"] = np.ascontiguousarray(x[b, half * IQ : (half + 1) * IQ, :].T).astype(bf)
        m["xkvT"] = np.ascontiguousarray(x[b].T).astype(bf)
        in_maps.append(m)

    res = run_bass_kernel_spmd(nc, in_maps, core_ids=list(range(8)))
    _CACHE["last_results"] = res
    _CACHE["last_in_maps"] = in_maps

    full = np.empty((b_, n_, d_), dtype=np.float32)
    for c in range(8):
        b, half = c // 2, c % 2
        full[b, half * IQ : (half + 1) * IQ, :] = res.results[c]["out"]
    return full


# revision 36
# speedup vs baseline: 1.0108x; 1.0021x over previous
"""Talking-heads attention on 8 Trainium2 NeuronCores.

Sharding: data-parallel over (batch b in 0..3) x (query half in 0..1) -> 8 cores.
Each core computes K/V for its full batch sequence (1024) and attention for its
512 query rows. No collectives.

Math notes (per core, all layouts transposed so contractions sit on partitions):
  - mix_pre (and SCALE) folded into Q per output-head g:
    qs_g[hd, i] = qT[hd, i] * scaleT[hd, g], scaleT = SCALE*mix_pre[h(hd), g]
    (host-precomputed), so dotsT_g[j, i] = sum_hd kT[hd, j] * qs_g[hd, i].
  - softmax over j (partitions) without max-subtraction (|dots| <~ 6, safe).
    The denominator never touches the PE: DVE tree-sums the 8 key chunks,
    GPSIMD partition_all_reduce sums across partitions (result broadcast to
    all partitions), DVE reciprocal + in-place scale of attnT.
  - mix_post folded into V: Vt_g[j, (g',d)] = mix_post[g, g'] * v[j, (g',d)];
    out2T[(g'd), i] += sum_j Vt_g[j, gd] * attnT_g[j, i] accumulated in PSUM
    over g (6 banks) while dots for g+2 stream (lag-2 software pipeline).
  - out = out2T.T @ Wout + bout (bf16), bias folded in as a K=1 accumulating
    matmul.
Scheduling notes: PE runs one long matmul stream (projections, 12x dots,
12x AV, out-proj); qs/Vt for the first heads and all PSUM->SBUF staging are
produced in phase 1 where ACT/DVE are idle, so the fill iterations of the
g-loop never stall on the scalar engines.
"""

import numpy as np

import concourse.bass_isa as bass_isa
import concourse.mybir as mybir
import concourse.tile as tile
from concourse import bacc
from concourse.bass_utils import run_bass_kernel_spmd

P = 128
DIM = 768
SEQ = 1024
IQ = 512            # query rows per core
H = 12
DH = 64
NC6 = DIM // P      # 6 chunks of the 768 dim
JC8 = SEQ // P      # 8 chunks of the key dim
SCALE = DH ** -0.5
F32 = mybir.dt.float32
BF16 = mybir.dt.bfloat16
EXP = mybir.ActivationFunctionType.Exp
ADD = mybir.AluOpType.add
MULT = mybir.AluOpType.mult

_CACHE = {}


def _build_nc():
    nc = bacc.Bacc("TRN2", target_bir_lowering=False, debug=False)

    xqT = nc.dram_tensor("xqT", [DIM, IQ], BF16, kind="ExternalInput")
    xkvT = nc.dram_tensor("xkvT", [DIM, SEQ], BF16, kind="ExternalInput")
    Wq = nc.dram_tensor("Wq", [DIM, DIM], BF16, kind="ExternalInput")
    Wk = nc.dram_tensor("Wk", [DIM, DIM], BF16, kind="ExternalInput")
    Wv = nc.dram_tensor("Wv", [DIM, DIM], BF16, kind="ExternalInput")
    Wout = nc.dram_tensor("Wout", [DIM, DIM], BF16, kind="ExternalInput")
    bout = nc.dram_tensor("bout", [1, DIM], BF16, kind="ExternalInput")
    scaleT_d = nc.dram_tensor("scaleT", [P, NC6 * H], F32, kind="ExternalInput")
    m2_d = nc.dram_tensor("m2", [1, H * H], F32, kind="ExternalInput")
    out = nc.dram_tensor("out", [IQ, DIM], F32, kind="ExternalOutput")

    r3 = lambda t: t.rearrange("(c p) e -> p c e", p=P)

    with tile.TileContext(nc) as tc:
        with (
            tc.tile_pool(name="persist", bufs=1) as pp,
            tc.tile_pool(name="attnp", bufs=4) as attnp,
            tc.tile_pool(name="qsp", bufs=4) as qsp,
            tc.tile_pool(name="vtp", bufs=3) as vtp,
            tc.tile_pool(name="sm1", bufs=1) as sm1,
            tc.tile_pool(name="sm2", bufs=2) as sm2,
        ):
            # ---- persistent tiles ----
            qT = pp.tile([P, NC6, IQ], BF16)
            kT = pp.tile([P, NC6, SEQ], BF16)
            V = pp.tile([P, JC8, DIM], BF16)     # [j-part, jc, (g,d)]
            Wout_sb = pp.tile([P, NC6, DIM], BF16)
            o2_sb = pp.tile([P, NC6, IQ], BF16)  # out2T staged for out-proj
            scaleT = pp.tile([P, NC6 * H], F32)  # SCALE*mix_pre expanded
            m2_sb = pp.tile([1, H * H], F32)
            m2bc = pp.tile([P, H * H], F32)      # mix_post bcast to all parts
            bout_sb = pp.tile([1, DIM], BF16)
            ones_row = pp.tile([1, P], BF16)
            nc.gpsimd.memset(ones_row[:], 1.0)

            s1 = sm1.tile([P, 4, IQ], BF16)
            s2 = sm1.tile([P, 2, IQ], BF16)

            qss, attnTs, vts = {}, {}, {}

            def make_qs(g):
                # qs(g) on ACT: per-partition scale fold
                qs = qsp.tile([P, NC6, IQ], BF16, tag="qs", name=f"qs{g}")
                for c in range(NC6):
                    nc.scalar.mul(
                        qs[:, c, :], qT[:, c, :],
                        scaleT[:, c * H + g : c * H + g + 1],
                    )
                qss[g] = qs

            def make_vt(g):
                # Vt(g) on DVE: mix_post column fold per 64-col group
                vt = vtp.tile([P, JC8, DIM], BF16, tag="vt", name=f"vt{g}")
                for gp in range(H):
                    nc.vector.tensor_scalar_mul(
                        vt[:, :, gp * DH : (gp + 1) * DH],
                        V[:, :, gp * DH : (gp + 1) * DH],
                        m2bc[:, g * H + gp : g * H + gp + 1],
                    )
                vts[g] = vt

            # ---- phase 1: input DMA + projections; also pre-produce the
            # qs/Vt operands the g-loop fill needs, while ACT/DVE are idle.
            with (
                tc.tile_pool(name="pin", bufs=1) as pin,
                tc.tile_pool(name="pj", bufs=2, space="PSUM") as pj,
                tc.tile_pool(name="pjv", bufs=2, space="PSUM") as pjv,
            ):
                xqT_sb = pin.tile([P, NC6, IQ], BF16)
                Wq_sb = pin.tile([P, NC6, DIM], BF16)
                xkvT_sb = pin.tile([P, NC6, SEQ], BF16)
                Wk_sb = pin.tile([P, NC6, DIM], BF16)
                Wv_sb = pin.tile([P, NC6, DIM], BF16)
                # input DMA: one queue, in consumption order; Wq/xqT land as
                # halves so the q projection can start on partial data.
                nc.sync.dma_start(Wq_sb[:, 0:3, :], r3(Wq)[:, 0:3, :])
                nc.sync.dma_start(xqT_sb[:, 0:3, :], r3(xqT)[:, 0:3, :])
                nc.sync.dma_start(Wq_sb[:, 3:6, :], r3(Wq)[:, 3:6, :])
                nc.sync.dma_start(xqT_sb[:, 3:6, :], r3(xqT)[:, 3:6, :])
                nc.sync.dma_start(xkvT_sb[:], r3(xkvT))
                nc.sync.dma_start(Wk_sb[:], r3(Wk))
                nc.sync.dma_start(Wv_sb[:], r3(Wv))
                nc.gpsimd.dma_start(scaleT[:], scaleT_d[:])
                nc.gpsimd.dma_start(m2_sb[:], m2_d[:])
                nc.gpsimd.dma_start(bout_sb[:], bout[:])
                nc.gpsimd.partition_broadcast(m2bc[:], m2_sb[:])

                # qT[e,i] = sum_f Wq[f,e] xqT[f,i]
                for ec in range(NC6):
                    ps = pj.tile([P, IQ], F32, tag="pjq")
                    for fc in range(NC6):
                        nc.tensor.matmul(
                            ps[:], Wq_sb[:, fc, ec * P : (ec + 1) * P],
                            xqT_sb[:, fc, :], start=(fc == 0), stop=(fc == NC6 - 1),
                        )
                    nc.scalar.copy(qT[:, ec, :], ps[:])

                # first two heads' qs while ACT is otherwise idle
                make_qs(0)
                make_qs(1)

                # kT[e,j]
                for ec in range(NC6):
                    for jh in range(2):
                        ps = pj.tile([P, IQ], F32, tag="pjq")
                        for fc in range(NC6):
                            nc.tensor.matmul(
                                ps[:], Wk_sb[:, fc, ec * P : (ec + 1) * P],
                                xkvT_sb[:, fc, jh * IQ : (jh + 1) * IQ],
                                start=(fc == 0), stop=(fc == NC6 - 1),
                            )
                        nc.scalar.copy(kT[:, ec, jh * IQ : (jh + 1) * IQ], ps[:])

                make_qs(2)

                # V[j, gd] = sum_f xkvT[f, j] Wv[f, gd]
                for jc in range(JC8):
                    psv = pjv.tile([P, DIM], F32, tag="pjv")
                    for ns, ne in ((0, 512), (512, DIM)):
                        for fc in range(NC6):
                            nc.tensor.matmul(
                                psv[:, ns:ne],
                                xkvT_sb[:, fc, jc * P : (jc + 1) * P],
                                Wv_sb[:, fc, ns:ne],
                                start=(fc == 0), stop=(fc == NC6 - 1),
                            )
                    nc.scalar.copy(V[:, jc, :], psv[:])

                # first two heads' Vt while DVE is otherwise idle
                make_vt(0)
                make_vt(1)

            # ---- phase 2: attention, lag-2 pipelined; softmax denom off-PE
            def produce(g, dspool):
                qs = qss.pop(g)
                if g >= 2:
                    make_vt(g)
                # dots(g) on PE + exp on ACT
                attnT = attnp.tile([P, JC8, IQ], BF16, tag="attnT")
                attnTs[g] = attnT
                for jc in range(JC8):
                    ds = dspool.tile([P, IQ], F32, tag="ds")
                    for c in range(NC6):
                        nc.tensor.matmul(
                            ds[:], kT[:, c, jc * P : (jc + 1) * P],
                            qs[:, c, :],
                            start=(c == 0), stop=(c == NC6 - 1),
                        )
                    nc.scalar.activation(attnT[:, jc, :], ds[:], EXP)
                # softmax denominator: DVE jc-tree + GPSIMD allreduce
                Ssum = sm2.tile([P, IQ], F32, tag="Ssum")
                rR = sm2.tile([P, IQ], F32, tag="rR")
                rRb = sm2.tile([P, IQ], BF16, tag="rRb")
                nc.vector.tensor_tensor(
                    s1[:], attnT[:, 0:4, :], attnT[:, 4:8, :], ADD
                )
                nc.vector.tensor_tensor(s2[:], s1[:, 0:2, :], s1[:, 2:4, :], ADD)
                nc.vector.tensor_tensor(Ssum[:], s2[:, 0, :], s2[:, 1, :], ADD)
                nc.gpsimd.partition_all_reduce(
                    Ssum[:], Ssum[:], channels=P, reduce_op=bass_isa.ReduceOp.add
                )
                nc.vector.reciprocal_approx_fast(rR[:], Ssum[:])
                nc.vector.tensor_copy(rRb[:], rR[:])
                nc.vector.tensor_tensor(
                    attnT[:], attnT[:],
                    rRb[:, None, :].to_broadcast((P, JC8, IQ)), MULT,
                )
                if g + 3 < H:
                    make_qs(g + 3)

            # fill iterations run before the o2 accumulator pool exists,
            # so the dots scratch gets 4 PSUM banks (no exp-lag stalls)
            with tc.tile_pool(name="pds0", bufs=4, space="PSUM") as pds0:
                produce(0, pds0)
                nc.gpsimd.dma_start(Wout_sb[:], r3(Wout))
                produce(1, pds0)

            with (
                tc.tile_pool(name="acc", bufs=1, space="PSUM") as acc,
                tc.tile_pool(name="pds", bufs=2, space="PSUM") as pds,
            ):
                o2ps = [
                    acc.tile([P, IQ], F32, tag=f"o2_{s}", name=f"o2_{s}")
                    for s in range(NC6)
                ]
                for it in range(2, H + 2):
                    if it < H:
                        produce(it, pds)
                    g2 = it - 2
                    last = g2 == H - 1
                    for s in range(NC6):
                        for jc in range(JC8):
                            nc.tensor.matmul(
                                o2ps[s][:],
                                vts[g2][:, jc, s * P : (s + 1) * P],
                                attnTs[g2][:, jc, :],
                                start=(g2 == 0 and jc == 0),
                                stop=(last and jc == JC8 - 1),
                            )
                        if last:
                            # drain each o2 bank as its final matmul lands
                            if s % 2 == 0:
                                nc.vector.tensor_copy(o2_sb[:, s, :], o2ps[s][:])
                            else:
                                nc.scalar.copy(o2_sb[:, s, :], o2ps[s][:])
                    del attnTs[g2], vts[g2]

            # ---- phase 3: output projection + bias via K=1 matmul ----
            with (
                tc.tile_pool(name="pj3", bufs=2, space="PSUM") as pj3,
                tc.tile_pool(name="ob", bufs=2) as ob,
            ):
                for isl in range(IQ // P):
                    fp = pj3.tile([P, DIM], F32, tag="fin")
                    for ns, ne in ((0, 512), (512, DIM)):
                        for c in range(NC6):
                            nc.tensor.matmul(
                                fp[:, ns:ne],
                                o2_sb[:, c, isl * P : (isl + 1) * P],
                                Wout_sb[:, c, ns:ne],
                                start=(c == 0), stop=False,
                            )
                        nc.tensor.matmul(
                            fp[:, ns:ne],
                            ones_row[:],
                            bout_sb[:, ns:ne],
                            start=False, stop=True,
                        )
                    osb = ob.tile([P, DIM], F32, tag="osb")
                    nc.scalar.copy(osb[:], fp[:])
                    eng = nc.gpsimd if isl % 2 == 0 else nc.sync
                    eng.dma_start(out[isl * P : (isl + 1) * P, :], osb[:])

    nc.compile()
    return nc


def kernel(x, Wq, Wkv, mix_pre, mix_post, Wout, bout):
    x = np.asarray(x, dtype=np.float32)
    Wq = np.asarray(Wq, dtype=np.float32)
    Wkv = np.asarray(Wkv, dtype=np.float32)
    mix_pre = np.asarray(mix_pre, dtype=np.float32)
    mix_post = np.asarray(mix_post, dtype=np.float32)
    Wout = np.asarray(Wout, dtype=np.float32)
    bout = np.asarray(bout, dtype=np.float32)

    if "nc" not in _CACHE:
        _CACHE["nc"] = _build_nc()
    nc = _CACHE["nc"]

    import ml_dtypes
    bf = ml_dtypes.bfloat16
    Wk = np.ascontiguousarray(Wkv[:, :DIM]).astype(bf)
    Wv = np.ascontiguousarray(Wkv[:, DIM:]).astype(bf)
    # scaleT[p, c*H+g] = SCALE * mix_pre[(c*128+p)//64, g]
    SM = SCALE * mix_pre[np.arange(DIM) // DH, :]          # [768, 12]
    scaleT = np.ascontiguousarray(
        SM.reshape(NC6, P, H).transpose(1, 0, 2).reshape(P, NC6 * H)
    ).astype(np.float32)
    shared = {
        "Wq": Wq.astype(bf), "Wk": Wk, "Wv": Wv, "Wout": Wout.astype(bf),
        "bout": np.ascontiguousarray(bout.reshape(1, DIM)).astype(bf),
        "scaleT": scaleT,
        "m2": np.ascontiguousarray(mix_post.reshape(1, H * H)),
    }
    b_, n_, d_ = x.shape
    in_maps = []
    for c in range(8):
        b, half = c // 2, c % 2
        m = dict(shared)
        m["xqT"] = np.ascontiguousarray(x[b, half * IQ : (half + 1) * IQ, :].T).astype(bf)
        m["xkvT"] = np.ascontiguousarray(x[b].T).astype(bf)
        in_maps.append(m)

    res = run_bass_kernel_spmd(nc, in_maps, core_ids=list(range(8)))
    _CACHE["last_results"] = res
    _CACHE["last_in_maps"] = in_maps

    full = np.empty((b_, n_, d_), dtype=np.float32)
    for c in range(8):
        b, half = c // 2, c % 2
        full[b, half * IQ : (half + 1) * IQ, :] = res.results[c]["out"]
    return full
